# revision 26
# baseline (speedup 1.0000x reference)
"""EntityAwareAttention Trainium2 kernel.

Problem (per batch b of B=2048):
    hid_e{1,2} = hidden[b, e{1,2}_idx[b]]                       # [600]
    e{1,2}_type = softmax(hid_e @ LT.T) @ LT                    # [600], T=3
    u1 = concat(hidden, pos1, pos2) @ W_hid.T                   # [128, 50]
    u2 = concat(hid_e1, e1_type, hid_e2, e2_type) @ W_ent.T     # [50]
    u = tanh(u1 + u2); scores = u @ v; alpha = softmax(scores)  # [128]
    z = alpha @ hidden[b]                                       # [600]

Sharding: pure data parallel over batch, 8 cores x 256 batches, weights
replicated, host concat of per-core outputs.

Kernel layout strategy (per core), ~315 us on HW:
  - Host pre-packs hidden -> bf16 and pos1|pos2 -> one bf16 [.,L,100]
    array (device pipeline never exceeds bf16 for these, so no accuracy
    is lost; halves HBM traffic and avoids the serialized SWDGE
    cast-DMA path). Hidden loads go on the Sync HWDGE queue, pos on the
    same queue behind them; z output + gathers on the GpSimd queue.
  - hidden+pos in [128 tok, 32 batch, 768] bf16 tiles (768 = 700
    features zero-padded to 6x128 chunks), rounds of 32 batches.
  - u1 needs feature-on-partition: 6 PE transposes per batch into bf16
    PSUM (ps_tp bufs=4 so transposes never stall on evacuation),
    evacuated per batch to fp8 hT tiles, alternating DVE / ACT.
  - u1 matmuls: fp8 DoubleRow (2 chunk-pairs per MM, K=256, N=512,
    weights zero-padded to 64-col pair stride) + one bf16 identity-lhsT
    matmul that broadcast-adds u2; tanh fused on ACT.
  - scores per-batch [tokens,1] matmuls; batched softmax in
    [32 batch, 128 tok] layout via one small PE transpose; alpha
    normalized in f32, transposed back bf16 for z.
  - z: per-batch matmul alpha.T @ hidden in PSUM (cheap: contraction
    happens inside the PE).
  - All small per-round PSUM tiles (scores, softmax transposes, z)
    share one [128, 480] f32 bank-packed tile, double-buffered; entity
    temporaries reuse the same tag. PSUM: tp 4 + u1 2 + rnd 2 = 8 banks.
  - Round tails (softmax+z+out) are emitted after the next round's
    front so the PE queue never drains.
  - entity path per 128-batch superbatch: indirect-DMA row gather,
    latent-type softmax via PE transposes, u2 = 20 accumulated matmuls.
"""

import numpy as np

B, L, H2, PP, A, T = 2048, 128, 600, 50, 50, 3
NCORES = 8
BC = B // NCORES  # 256 batches per core
SB = 128          # superbatch for the entity/u2 pipeline
ROUND = 32        # batches per softmax/z round
GROUP = 4         # batches per u1 matmul group (N = 4*128 = 512)
F = H2 + 2 * PP   # 700 concat features
FPAD = 768        # padded to 6 x 128
NCH = 6           # feature chunks of 128
EPAD = 640        # 600-dim entity vectors padded to 5 x 128
ECH = 5

_CACHE = {}


def _build_bass(skip=()):
    import os
    skip = set(skip) or set(
        x for x in os.environ.get("KBENCH_SKIP", "").split(",") if x
    )
    import concourse.bass as bass
    import concourse.bacc as bacc
    import concourse.tile as tile
    from concourse import mybir
    from concourse.masks import make_identity

    f32 = mybir.dt.float32
    bf16 = mybir.dt.bfloat16
    fp8 = mybir.dt.float8e4
    i32 = mybir.dt.int32
    DR = mybir.MatmulPerfMode.DoubleRow
    AF = mybir.ActivationFunctionType
    AX = mybir.AxisListType

    nc = bacc.Bacc("TRN2", debug=False, target_bir_lowering=False)

    hid_d = nc.dram_tensor("hidden", [BC, L, H2], bf16, kind="ExternalInput").ap()
    pos_d = nc.dram_tensor("pos", [BC, L, 2 * PP], bf16, kind="ExternalInput").ap()
    e1r_d = nc.dram_tensor("e1rows", [BC, 1], i32, kind="ExternalInput").ap()
    e2r_d = nc.dram_tensor("e2rows", [BC, 1], i32, kind="ExternalInput").ap()
    whid_d = nc.dram_tensor("w_hid", [A, F], f32, kind="ExternalInput").ap()
    went_d = nc.dram_tensor("w_ent", [A, 4 * H2], f32, kind="ExternalInput").ap()
    lt_d = nc.dram_tensor("latent", [T, H2], f32, kind="ExternalInput").ap()
    v_d = nc.dram_tensor("v", [A, 1], f32, kind="ExternalInput").ap()
    z_d = nc.dram_tensor(
        "z", [BC // ROUND, 128, ECH, ROUND], f32, kind="ExternalOutput"
    ).ap()

    hid_flat = hid_d.rearrange("b l d -> (b l) d")

    with tile.TileContext(nc) as tc:
        with (
            tc.tile_pool(name="const", bufs=1) as const,
            tc.tile_pool(name="hp_pool", bufs=3) as hp_pool,
            tc.tile_pool(name="ht_pool", bufs=4) as ht_pool,
            tc.tile_pool(name="u_pool", bufs=3) as u_pool,
            tc.tile_pool(name="ent_pool", bufs=2) as ent_pool,
            tc.tile_pool(name="small", bufs=4) as small,
            tc.tile_pool(name="zs_pool", bufs=2) as zs_pool,
            tc.tile_pool(name="ps_tp", bufs=4, space="PSUM") as ps_tp,
            tc.tile_pool(name="ps_u1", bufs=2, space="PSUM") as ps_u1,
            tc.tile_pool(name="ps_rnd", bufs=2, space="PSUM") as ps_rnd,

        ):
            # ---------------- one-time constants ----------------
            id_f32 = const.tile([128, 128], f32)
            make_identity(nc, id_f32[:, :])
            id_bf = const.tile([128, 128], bf16)
            nc.vector.tensor_copy(id_bf[:, :], id_f32[:, :])

            # W_hid -> transposed bf16 chunks [128, 6, 50]
            whid_sb = const.tile([A, FPAD], f32)
            nc.gpsimd.memset(whid_sb[:, :], 0.0)
            nc.gpsimd.dma_start(out=whid_sb[:, 0:F], in_=whid_d)
            whT_ps = ps_u1.tile([128, NCH, 64], f32, tag="u1like")
            for c in range(NCH):
                nc.tensor.transpose(
                    whT_ps[:, c, 0:A], whid_sb[:, c * 128:(c + 1) * 128],
                    id_f32[0:A, 0:A],
                )
            whidT = const.tile([128, NCH, 64], fp8)
            nc.gpsimd.memset(whidT[:, :, :], 0.0)
            nc.vector.tensor_copy(whidT[:, :, 0:A], whT_ps[:, :, 0:A])

            # W_ent -> padded [50, 4*640] then transposed bf16 [128, 20, 50]
            went_sb = const.tile([A, 4 * EPAD], f32)
            nc.gpsimd.memset(went_sb[:, :], 0.0)
            nc.gpsimd.dma_start(
                out=went_sb[:, :].rearrange("a (q d) -> a q d", q=4)[:, :, 0:H2],
                in_=went_d.rearrange("a (q d) -> a q d", q=4),
            )
            wentT = const.tile([128, 4 * ECH, A], bf16)
            for quarter in range(4):
                weT_ps = ps_u1.tile([128, ECH, 64], f32, tag="u1like")
                for cc in range(ECH):
                    c = quarter * ECH + cc
                    nc.tensor.transpose(
                        weT_ps[:, cc, 0:A],
                        went_sb[:, c * 128:(c + 1) * 128],
                        id_f32[0:A, 0:A],
                    )
                nc.vector.tensor_copy(
                    wentT[:, quarter * ECH:(quarter + 1) * ECH, :],
                    weT_ps[:, :, 0:A],
                )

            # latent_types: padded f32 [3, 640], bf16 copy, transposed chunks
            lt_sb = const.tile([T, EPAD], f32)
            nc.gpsimd.memset(lt_sb[:, :], 0.0)
            nc.gpsimd.dma_start(out=lt_sb[:, 0:H2], in_=lt_d)
            lt16 = const.tile([T, H2], bf16)
            nc.gpsimd.dma_start(out=lt16[:, :], in_=lt_d)
            ltT_ps = ps_u1.tile([128, ECH, 4], f32, tag="u1like")
            for c in range(ECH):
                nc.tensor.transpose(
                    ltT_ps[:, c, 0:T], lt_sb[:, c * 128:(c + 1) * 128],
                    id_f32[0:T, 0:T],
                )
            ltT = const.tile([128, ECH, T], bf16)
            nc.vector.tensor_copy(ltT[:, :, :], ltT_ps[:, :, 0:T])

            v16 = const.tile([A, 1], bf16)
            nc.gpsimd.dma_start(out=v16[:, :], in_=v_d)

            chunks = [(c * 128, min(128, F - c * 128)) for c in range(NCH)]

            def entity_block(s):
                """Gather + latent-type + u2 for superbatch s (128 batches).
                Returns u2sb [50, 128] f32."""
                ernd = ps_rnd.tile([128, 480], f32, tag="rnd")
                etT_list = []
                for rows_d in (e1r_d, e2r_d):
                    rows = ent_pool.tile([SB, 1], i32, tag="rows")
                    nc.gpsimd.dma_start(
                        out=rows[:, :], in_=rows_d[s * SB:(s + 1) * SB, :]
                    )
                    ent = ent_pool.tile([SB, EPAD], bf16, tag="ent")
                    nc.gpsimd.memset(ent[:, H2:EPAD], 0.0)
                    nc.gpsimd.indirect_dma_start(
                        out=ent[:, 0:H2],
                        out_offset=None,
                        in_=hid_flat,
                        in_offset=bass.IndirectOffsetOnAxis(ap=rows[:, 0:1], axis=0),
                    )
                    # transpose gathered entities -> entT [128, 5, 128] bf16
                    entT = ent_pool.tile([128, ECH, SB], bf16, tag="entT")
                    for c in range(ECH):
                        tp = ernd[:, 352:480].bitcast(bf16)[:, 0:SB]
                        nc.tensor.transpose(
                            tp[:, :], ent[:, c * 128:(c + 1) * 128], id_bf[:, :]
                        )
                        nc.vector.tensor_copy(entT[:, c, :], tp[:, :])
                    # latent-type logits: [3, 128] = sum_c ltT_c.T @ entT_c
                    lg_ps = ernd[0:T, 352:480]
                    for c in range(ECH):
                        nc.tensor.matmul(
                            lg_ps[:, :], lhsT=ltT[:, c, :], rhs=entT[:, c, :],
                            start=(c == 0), stop=(c == ECH - 1),
                        )
                    lgT_sb = ent_pool.tile([T, SB], f32, tag="lgT")
                    nc.vector.tensor_copy(lgT_sb[:, :], lg_ps[:, :])
                    lg2_ps = ernd[0:SB, 352:352 + T]
                    nc.tensor.transpose(lg2_ps[:, :], lgT_sb[:, :], id_f32[0:T, 0:T])
                    expl = ent_pool.tile([SB, T], f32, tag="expl")
                    nc.scalar.activation(expl[:, :], lg2_ps[:, :], AF.Exp)
                    ssum = ent_pool.tile([SB, 1], f32, tag="ssum")
                    nc.vector.reduce_sum(ssum[:, :], expl[:, :], axis=AX.X)
                    srec = ent_pool.tile([SB, 1], f32, tag="srec")
                    nc.vector.reciprocal(srec[:, :], ssum[:, :])
                    attw = ent_pool.tile([SB, T], f32, tag="attw")
                    nc.vector.tensor_scalar_mul(attw[:, :], expl[:, :], srec[:, 0:1])
                    awT_ps = ernd[0:T, 352:480]
                    nc.tensor.transpose(awT_ps[:, :], attw[:, :], id_f32[:, :])
                    awT = ent_pool.tile([T, SB], bf16, tag="awT_sb")
                    nc.vector.tensor_copy(awT[:, :], awT_ps[:, :])
                    # e_type = attw @ LT : [128, 600] (f32 psum, bf16 sbuf)
                    et_lo = ps_u1.tile([SB, 512], f32, tag="u1like")
                    et_hi = ernd[0:SB, 352:480]
                    nc.tensor.matmul(
                        et_lo[:, :], lhsT=awT[:, :], rhs=lt16[:, 0:512],
                        start=True, stop=True,
                    )
                    nc.tensor.matmul(
                        et_hi[:, 0:H2 - 512], lhsT=awT[:, :], rhs=lt16[:, 512:H2],
                        start=True, stop=True,
                    )
                    et = ent_pool.tile([SB, EPAD], bf16, tag="et_sb")
                    nc.gpsimd.memset(et[:, H2:EPAD], 0.0)
                    nc.scalar.activation(et[:, 0:512], et_lo[:, :], AF.Copy)
                    nc.scalar.activation(et[:, 512:H2], et_hi[:, 0:H2 - 512], AF.Copy)
                    # transpose e_type -> etT [128, 5, 128] bf16
                    etT = ent_pool.tile([128, ECH, SB], bf16, tag="etT")
                    etT_ps = ps_tp.tile([128, ECH, SB], bf16, tag="tp")
                    for c in range(ECH):
                        nc.tensor.transpose(
                            etT_ps[:, c, :], et[:, c * 128:(c + 1) * 128],
                            id_bf[:, :],
                        )
                    nc.vector.tensor_copy(etT[:, :, :], etT_ps[:, :, :])
                    etT_list.append((entT, etT))

                # u2T [50, 128] = sum over 20 chunks W_entT_c.T @ srcT_c
                order = [
                    etT_list[0][0], etT_list[0][1],
                    etT_list[1][0], etT_list[1][1],
                ]
                u2_ps = ernd[0:A, 352:480]
                k = 0
                for q in range(4):
                    for c in range(ECH):
                        nc.tensor.matmul(
                            u2_ps[:, :],
                            lhsT=wentT[:, q * ECH + c, :],
                            rhs=order[q][:, c, :],
                            start=(k == 0), stop=(k == 19),
                        )
                        k += 1
                u2sb16 = ent_pool.tile([A, SB], bf16, tag="u2sb")
                nc.vector.tensor_copy(u2sb16[:, :], u2_ps[:, :])
                return u2sb16

            hp_cache = {}

            def load_round(s, r):
                b0 = s * SB + r * ROUND
                hp = hp_pool.tile([L, ROUND, FPAD], bf16, tag="hp")
                nc.gpsimd.memset(hp[:, :, F:FPAD], 0.0)
                nc.sync.dma_start(
                    out=hp[:, :, 0:H2],
                    in_=hid_d[b0:b0 + ROUND].rearrange("i l d -> l i d"),
                )
                nc.sync.dma_start(
                    out=hp[:, :, H2:F],
                    in_=pos_d[b0:b0 + ROUND].rearrange("i l d -> l i d"),
                )
                hp_cache[(s, r)] = hp

            def do_front(s, r, u2sb16):
                b0 = s * SB + r * ROUND  # first batch of round (core-local)
                if (s, r) not in hp_cache:
                    load_round(s, r)
                hp = hp_cache.pop((s, r))

                rnd = ps_rnd.tile([128, 480], f32, tag="rnd")
                sc_ps = rnd[:, 0:ROUND]
                for g in range(ROUND // GROUP):
                    hT = ht_pool.tile([128, NCH, GROUP * L], fp8, tag="hT")
                    if "tp" in skip:
                        continue
                    for j in range(GROUP):
                        bl = g * GROUP + j
                        tp = ps_tp.tile([128, NCH * L], bf16, tag="tp")
                        for c in range(NCH):
                            nc.tensor.transpose(
                                tp[:, c * L:(c + 1) * L],
                                hp[:, bl, c * 128:(c + 1) * 128],
                                id_bf[:, :],
                            )
                        tpv = tp[:, :].rearrange("p (c t) -> p c t", c=NCH)
                        if j % 2 == 0:
                            nc.vector.tensor_copy(
                                hT[:, :, j * L:(j + 1) * L], tpv[:, :, :]
                            )
                        else:
                            nc.scalar.activation(
                                hT[:, :, j * L:(j + 1) * L], tpv[:, :, :],
                                AF.Copy,
                            )
                    if "u1" in skip:
                        continue
                    u1_ps = ps_u1.tile([A, GROUP * L], f32, tag="u1like")
                    hTd = hT[:, :, :].rearrange("p (h two) t -> p h two t", two=2)
                    whd = whidT[:, :, :].rearrange("p (h two) a -> p h two a", two=2)
                    for c in range(NCH // 2):
                        nc.tensor.matmul(
                            u1_ps[:, :],
                            lhsT=whd[:, c, :, 0:A], rhs=hTd[:, c, :, :],
                            perf_mode=DR,
                            start=(c == 0), stop=False,
                        )
                    # += u2 (broadcast over tokens) via identity-lhsT matmul:
                    # out[a, n] += sum_k I[k, a] * u2[k, batch(n)]
                    b0r = r * ROUND + g * GROUP
                    u2r = u2sb16[:, b0r:b0r + GROUP]
                    u2b = bass.AP(
                        tensor=u2r.tensor, offset=u2r.offset,
                        ap=[u2r.ap[0], u2r.ap[1], [0, L]],
                    )
                    nc.tensor.matmul(
                        u1_ps[:, :], lhsT=id_bf[0:A, 0:A], rhs=u2b,
                        start=False, stop=True,
                    )
                    uT = u_pool.tile([A, GROUP * L], bf16, tag="uT")
                    nc.scalar.activation(uT[:, :], u1_ps[:, :], AF.Tanh)
                    for j in range(GROUP):
                        bl = g * GROUP + j
                        nc.tensor.matmul(
                            sc_ps[:, bl:bl + 1],
                            lhsT=uT[:, j * L:(j + 1) * L],
                            rhs=v16[:, 0:1],
                            start=True, stop=True,
                        )

                return (s, r, rnd, sc_ps, hp)

            def do_tail(state):
                if "sm" in skip or "u1" in skip or "tp" in skip:
                    return
                s, r, rnd, sc_ps, hp = state
                # batched softmax over tokens for the 32 batches
                scT_sb = small.tile([L, ROUND], f32, tag="scT_sb")
                nc.vector.tensor_copy(scT_sb[:, :], sc_ps[:, :])
                sc2_ps = rnd[0:ROUND, 32:160]
                nc.tensor.transpose(sc2_ps[:, :], scT_sb[:, :], id_f32[:, :])
                exps = small.tile([ROUND, L], bf16, tag="exps")
                esum = small.tile([ROUND, 1], f32, tag="esum")
                nc.scalar.activation(
                    exps[:, :], sc2_ps[:, :], AF.Exp, accum_out=esum[:, :])
                erec = small.tile([ROUND, 1], f32, tag="erec")
                nc.vector.reciprocal(erec[:, :], esum[:, :])
                alph = small.tile([ROUND, L], f32, tag="alph")
                nc.vector.tensor_scalar_mul(alph[:, :], exps[:, :], erec[:, 0:1])
                aT_ps = rnd[:, 160:160 + ROUND]
                nc.tensor.transpose(aT_ps[:, :], alph[:, :], id_f32[0:ROUND, 0:ROUND])
                alphaT = small.tile([L, ROUND], bf16, tag="alphaT")
                nc.vector.tensor_copy(alphaT[:, :], aT_ps[:, :])

                # z transposed: zT[d, b] = sum_l hp[l, b, d] * alpha[l, b]
                # (feature chunk c=4 spans [512:640); cols 600:640 are junk the
                # host drops)
                if "z" in skip:
                    return
                zt_ps = rnd[:, 192:352].rearrange("p (c r) -> p c r", c=ECH)
                for bl in range(ROUND):
                    for c in range(ECH):
                        nc.tensor.matmul(
                            zt_ps[:, c, bl:bl + 1],
                            lhsT=hp[:, bl, c * 128:(c + 1) * 128],
                            rhs=alphaT[:, bl:bl + 1],
                            start=True, stop=True,
                        )
                zt_sb = zs_pool.tile([128, ECH, ROUND], f32, tag="zt_sb")
                nc.vector.tensor_copy(zt_sb[:, :, :], zt_ps[:, :, :])
                ri = s * (SB // ROUND) + r
                nc.gpsimd.dma_start(out=z_d[ri], in_=zt_sb[:, :, :])

            load_round(0, 0)
            load_round(0, 1)
            pend = None
            for s in range(BC // SB):
                if "ent" in skip:
                    u2sb_t = ent_pool.tile([A, SB], bf16, tag="u2sb")
                    nc.gpsimd.memset(u2sb_t[:, :], 0.0)
                    u2sb16 = u2sb_t
                else:
                    u2sb16 = entity_block(s)
                for r in range(SB // ROUND):
                    st = do_front(s, r, u2sb16)
                    if pend is not None:
                        do_tail(pend)
                    pend = st
            if pend is not None:
                do_tail(pend)

    nc.compile()
    return nc


def _get_nc():
    if "nc" not in _CACHE:
        _CACHE["nc"] = _build_bass()
    return _CACHE["nc"]


def make_in_maps(inputs):
    import ml_dtypes
    bf = ml_dtypes.bfloat16
    hidden = np.ascontiguousarray(np.asarray(inputs["hidden"]).astype(bf))
    pos = np.ascontiguousarray(np.concatenate(
        [np.asarray(inputs["pos1_emb"]), np.asarray(inputs["pos2_emb"])],
        axis=-1).astype(bf))
    e1 = np.asarray(inputs["entity1_idx"]).astype(np.int64)
    e2 = np.asarray(inputs["entity2_idx"]).astype(np.int64)
    w_hid = np.ascontiguousarray(np.asarray(inputs["W_hid"], dtype=np.float32))
    w_ent = np.ascontiguousarray(np.asarray(inputs["W_ent"], dtype=np.float32))
    lt = np.ascontiguousarray(np.asarray(inputs["latent_types"], dtype=np.float32))
    v = np.ascontiguousarray(np.asarray(inputs["v"], dtype=np.float32))

    loc = np.arange(BC, dtype=np.int64) * L
    in_maps = []
    for c in range(NCORES):
        sl = slice(c * BC, (c + 1) * BC)
        in_maps.append({
            "hidden": hidden[sl],
            "pos": pos[sl],
            "e1rows": np.ascontiguousarray(
                (loc + e1[sl]).astype(np.int32)[:, None]),
            "e2rows": np.ascontiguousarray(
                (loc + e2[sl]).astype(np.int32)[:, None]),
            "w_hid": w_hid,
            "w_ent": w_ent,
            "latent": lt,
            "v": v,
        })
    return in_maps


def unshard_z(zt):
    # zt: [BC//ROUND, 128, ECH, ROUND] with z[r*ROUND+q, c*128+p] = zt[r,p,c,q]
    z = np.transpose(np.asarray(zt), (0, 3, 2, 1)).reshape(BC, ECH * 128)
    return z[:, :H2]


def kernel(**inputs):
    from concourse.bass_utils import run_bass_kernel_spmd

    nc = _get_nc()
    in_maps = make_in_maps(inputs)
    res = run_bass_kernel_spmd(nc, in_maps, core_ids=list(range(NCORES)))
    _CACHE["last_res"] = res
    outs = [unshard_z(r["z"]) for r in res.results]
    return np.concatenate(outs, axis=0).astype(np.float32)

